# revision 19
# baseline (speedup 1.0000x reference)
"""Trainium2 Bass kernel for nn_MultiHeadDuelingDQN (8-core SPMD), v2.

Model (B=256, STATE=26240, H=512, R=4000, N=64 heads, M=10):
    h  = relu(relu(x@W1+b1)@W2+b2)
    q_cache = h@Wvc+bvc + (h@Wac+bac) - mean_R(h@Wac+bac)
    q_assoc = per-head dueling over M (local means)
    q_rec   = S - mean_R(S),  S = h @ W_sum + sum_n bru[n],
              W_sum = sum_n Wru[n]  (exact rewrite; see v1 notes)

v2 redesign vs the f32 baseline (344-438us):
  - All large streams are bf16 (Wru, W1, x, W2, Wac, aug): DMA floor drops
    from ~80MB to ~41MB per core; matmuls run at bf16 PE rate.
  - Host pre-transposes x and lays out W1 so fc1 computes h1T = W1.T x.T
    directly -- zero on-chip transposes (v1 had 60+ PE transposes + copies).
  - Trunk exchange is ReduceScatter(f32, +) then relu+bias+cast and
    AllGather(bf16): the v1 AllToAll measured 96us+39us skew; AG measured
    16.7us. fc2 is then computed replicated (2us of PE).
  - Wru stream: 16 supertiles of [128, 16*500] bf16 (2MB each, head-blocked
    layout) on the sync HWDGE ring ONLY. DVE tensor_reduce has no 2x uop
    (1 elem/cycle cap), so the head-sum is an IN-PLACE pairwise add-tree of
    tensor_tensor ops: bf16 step-1 adds run in 2x_1P mode (2 elem/cycle).
    L1-L3 halve in place inside the supertile (bf16), L4 emits f32, cross-
    supertile adds in f32, final add per k-chunk emits bf16 W_sum.
  - Collective bounce READBACKS are on gpsimd (SWDGE) so HWDGE lanes never
    chain behind ncfw latency (v1's 139us Sync stall).
  - The R-mean exchange is split: adv_c row-sums AllGather fires right after
    the cache head (hidden under the stream; q_cache finalized + written
    early), S row-sums AllGather is the only tail collective.

kernel(**inputs) takes full unsharded inputs, returns full [256, 8640].
"""
import os
os.environ.setdefault("NEURON_RT_DBG_RDH_CC", "0")

import numpy as np
import ml_dtypes

import concourse.bass as bass
import concourse.mybir as mybir
import concourse.tile as tile
from concourse import bacc
from concourse import bass_utils
from concourse.bass import ts

NC = 8
B, H, STATE, R, NH, M = 256, 512, 26240, 4000, 64, 10
KPC_RAW = STATE // NC          # 3280
KCH = 26                       # k-chunks of 128 per core (padded)
KPC = KCH * 128                # 3328
RPC = R // NC                  # 500
HPC = NH // NC                 # 8 heads per core
HS = H // NC                   # 64 h1 rows per core after ReduceScatter
AUG = HPC * (M + 1) + 1        # 89 = [8x(10 adv + 1 val)] + value_c
GRP = 8                        # heads per wru supertile
NGRP = NH // GRP               # 4 supertiles per k-chunk
F32 = mybir.dt.float32
BF16 = mybir.dt.bfloat16
RELU = mybir.ActivationFunctionType.Relu
COPY = mybir.ActivationFunctionType.Copy
IDENT = mybir.ActivationFunctionType.Identity
ADD = mybir.AluOpType.add
RG = [list(range(NC))]


def build_program(wru_bufs=8):
    nc = bacc.Bacc("TRN2", target_bir_lowering=False, debug=False, num_devices=NC)

    # ---- per-core inputs (host-packed layouts, see make_in_maps) ----
    xt = nc.dram_tensor("xt", [128, KCH * B], BF16, kind="ExternalInput").ap()
    w1 = nc.dram_tensor("w1", [128, KCH * H], BF16, kind="ExternalInput").ap()
    w2 = nc.dram_tensor("w2", [128, 4 * H], BF16, kind="ExternalInput").ap()
    b1s = nc.dram_tensor("b1s", [HS, 1], F32, kind="ExternalInput").ap()
    b2c = nc.dram_tensor("b2c", [128, 4], F32, kind="ExternalInput").ap()
    wac = nc.dram_tensor("wac", [128, 4 * RPC], BF16, kind="ExternalInput").ap()
    bac = nc.dram_tensor("bac", [1, RPC], BF16, kind="ExternalInput").ap()
    # [kc, grp, p, r*GRP+gi] = Wru[grp*GRP+gi, kc*128+p, r0+r]
    wru = nc.dram_tensor("wru", [4, NGRP, 128, GRP * RPC], BF16,
                         kind="ExternalInput").ap()
    bru = nc.dram_tensor("bru", [NH, RPC], BF16, kind="ExternalInput").ap()
    aug_w = nc.dram_tensor("aug_w", [128, 4 * AUG], BF16, kind="ExternalInput").ap()
    aug_b = nc.dram_tensor("aug_b", [1, AUG], BF16, kind="ExternalInput").ap()

    out_cache = nc.dram_tensor("out_cache", [B, RPC], F32, kind="ExternalOutput").ap()
    out_rec = nc.dram_tensor("out_rec", [B, RPC], F32, kind="ExternalOutput").ap()
    out_assoc = nc.dram_tensor("out_assoc", [B, HPC * M], F32, kind="ExternalOutput").ap()

    with tile.TileContext(nc) as tc:
        with (
            tc.tile_pool(name="cst", bufs=1) as cst,
            tc.tile_pool(name="sb", bufs=1) as sb,
            tc.tile_pool(name="wrup", bufs=wru_bufs) as wrup,
            tc.tile_pool(name="s500p", bufs=2) as s500p,
            tc.tile_pool(name="psfc", bufs=4, space="PSUM") as psfc,
            tc.tile_pool(name="psh", bufs=4, space="PSUM") as psh,
            tc.tile_pool(name="dram", bufs=1, space="DRAM") as dram,
        ):
            ones1 = cst.tile([1, 128], BF16, tag="ones1")
            nc.vector.memset(ones1, 1.0)
            ones64 = cst.tile([64, 128], BF16, tag="ones64")
            nc.vector.memset(ones64, 1.0)

            # ---- weight/input loads: scalar (ACT) HWDGE ring, all fast ----
            # xt/w1 in 4 chunk-tile pairs (separate tiles: Tile dependency
            # tracking is tile-granular, so slices of one big tile would make
            # every fc1 matmul wait for the last chunk).
            KGROUPS = [(0, 7), (7, 7), (14, 6), (20, 6)]
            xt_g, w1_g = [], []
            for gi, (g0, gl) in enumerate(KGROUPS):
                xg = sb.tile([128, gl * B], BF16, tag=f"xt_g{gi}", name=f"xt_g{gi}")
                nc.scalar.dma_start(xg, xt[:, g0 * B:(g0 + gl) * B])
                wg = sb.tile([128, gl * H], BF16, tag=f"w1_g{gi}", name=f"w1_g{gi}")
                nc.scalar.dma_start(wg, w1[:, g0 * H:(g0 + gl) * H])
                xt_g.append(xg)
                w1_g.append(wg)
            w2_sb = sb.tile([128, 4 * H], BF16, tag="w2_sb")
            nc.scalar.dma_start(w2_sb, w2)
            wac_sb = sb.tile([128, 4 * RPC], BF16, tag="wac_sb")
            nc.scalar.dma_start(wac_sb, wac)
            aug_sb = cst.tile([128, 4 * AUG], BF16, tag="aug_sb")
            nc.scalar.dma_start(aug_sb, aug_w)
            augb_sb = cst.tile([1, AUG], BF16, tag="augb_sb")
            nc.scalar.dma_start(augb_sb, aug_b)
            bac_sb = cst.tile([1, RPC], BF16, tag="bac_sb")
            nc.scalar.dma_start(bac_sb, bac)
            bru_sb = sb.tile([64, RPC], BF16, tag="bru_sb")
            nc.scalar.dma_start(bru_sb, bru)
            b1s_sb = cst.tile([HS, 1], F32, tag="b1s_sb")
            nc.scalar.dma_start(b1s_sb, b1s)
            b2c_sb = cst.tile([128, 4], F32, tag="b2c_sb")
            nc.scalar.dma_start(b2c_sb, b2c)

            # ---- Wru stream (sync HWDGE ring only) + DVE add-tree ----
            # Dummy pool-slot writes gate the first wru_bufs wru DMAs on the
            # trunk chunk loads, so xt/w1 get full HBM bandwidth first and
            # fc1 can start ~10us in; the wru stream has ~25us of slack.
            gates = [xt_g[0], w1_g[0], xt_g[1], w1_g[1],
                     xt_g[2], w1_g[2], xt_g[3], w1_g[3]]
            for i in range(min(wru_bufs, len(gates))):
                dmy = wrup.tile([1, 2], BF16, tag="wru", name=f"wru_gate{i}")
                nc.vector.tensor_copy(dmy, gates[i][0:1, 0:2])
            accf = [sb.tile([128, RPC], F32, tag=f"accf{k}", name=f"accf{k}")
                    for k in range(4)]
            accb = [sb.tile([128, RPC], BF16, tag=f"accb{k}", name=f"accb{k}")
                    for k in range(4)]
            with nc.allow_low_precision(reason="bf16 2x-mode add tree; "
                                        "f32 from L3 up, <0.3% on W_sum"):
                for kc in range(4):
                    for g in range(NGRP):
                        wt = wrup.tile([128, GRP * RPC], BF16, tag="wru",
                                       name=f"wru_t{kc}_{g}")
                        nc.sync.dma_start(wt, wru[kc, g])
                        # in-place halving tree (bf16, step-1 -> 2x_1P mode)
                        for half in (2000, 1000):
                            nc.vector.tensor_tensor(
                                out=wt[:, 0:half], in0=wt[:, 0:half],
                                in1=wt[:, half:2 * half], op=ADD)
                        if g == 0:
                            nc.vector.tensor_tensor(
                                out=accf[kc], in0=wt[:, 0:RPC],
                                in1=wt[:, RPC:2 * RPC], op=ADD)
                        else:
                            s500 = s500p.tile([128, RPC], F32, tag="s500",
                                              name=f"s500_{kc}_{g}")
                            nc.vector.tensor_tensor(
                                out=s500, in0=wt[:, 0:RPC],
                                in1=wt[:, RPC:2 * RPC], op=ADD)
                            if g < NGRP - 1:
                                nc.vector.tensor_add(accf[kc], accf[kc], s500)
                            else:
                                nc.vector.tensor_tensor(
                                    out=accb[kc], in0=accf[kc], in1=s500, op=ADD)

            # ---- fc1: h1T partial = W1_slice.T @ x_slice.T  (4 psum banks) ----
            ps1 = [psfc.tile([128, B], F32, tag="fc", name=f"ps1_{jc}")
                   for jc in range(4)]
            for gi, (g0, gl) in enumerate(KGROUPS):
                for k in range(gl):
                    kc = g0 + k
                    for jc in range(4):
                        nc.tensor.matmul(
                            ps1[jc],
                            w1_g[gi][:, k * H + jc * 128: k * H + (jc + 1) * 128],
                            xt_g[gi][:, k * B:(k + 1) * B],
                            start=(kc == 0), stop=(kc == KCH - 1))

            # bounce partials to DRAM (one 512KB write on scalar ring)
            rs_in = dram.tile([H, B], F32, tag="rs_in")
            h1p = sb.tile([128, 4 * B], F32, tag="h1p")
            for jc in range(4):
                nc.scalar.copy(h1p[:, jc * B:(jc + 1) * B], ps1[jc])
            nc.gpsimd.dma_start(
                rs_in.rearrange("(jc p) b -> p jc b", p=128),
                h1p.rearrange("p (jc b) -> p jc b", b=B))

            # ReduceScatter(+): each core gets its 64 summed h1T rows
            rs_out = dram.tile([HS, B], F32, tag="rs_out")
            nc.gpsimd.collective_compute(
                "ReduceScatter", ADD, replica_groups=RG,
                ins=[rs_in.opt()], outs=[rs_out.opt()])
            rsloc = sb.tile([HS, B], F32, tag="rsloc")
            nc.gpsimd.dma_start(rsloc, rs_out)
            h1loc = sb.tile([HS, B], BF16, tag="h1loc")
            nc.scalar.activation(h1loc, rsloc, RELU, bias=b1s_sb, scale=1.0)

            # AllGather (bf16) -> full h1T, read back as 4 [128,256] chunks
            ag_in = dram.tile([HS, B], BF16, tag="ag_in")
            nc.gpsimd.dma_start(ag_in, h1loc)
            ag_out = dram.tile([H, B], BF16, tag="ag_out")
            nc.gpsimd.collective_compute(
                "AllGather", mybir.AluOpType.bypass, replica_groups=RG,
                ins=[ag_in.opt()], outs=[ag_out.opt()])
            h1T = sb.tile([128, 4 * B], BF16, tag="h1T")
            nc.gpsimd.dma_start(
                h1T.rearrange("p (kc b) -> p kc b", b=B),
                ag_out.rearrange("(kc p) b -> p kc b", p=128))

            # ---- fc2 (replicated): h2T = relu(W2.T @ h1T + b2) -> hT bf16 ----
            ps2 = [psfc.tile([128, B], F32, tag="fc", name=f"ps2_{jc}")
                   for jc in range(4)]
            for kc in range(4):
                for jc in range(4):
                    nc.tensor.matmul(
                        ps2[jc],
                        w2_sb[:, kc * H + jc * 128: kc * H + (jc + 1) * 128],
                        h1T[:, kc * B:(kc + 1) * B],
                        start=(kc == 0), stop=(kc == 3))
            hT = sb.tile([128, 4 * B], BF16, tag="hT")
            for jc in range(4):
                nc.scalar.activation(hT[:, jc * B:(jc + 1) * B], ps2[jc],
                                     RELU, bias=b2c_sb[:, jc:jc + 1], scale=1.0)

            # ---- assoc heads: augmented [adv | val | value_c] ----
            junkA = sb.tile([128, M], F32, tag="junkA")
            value_sb = []
            for bt in range(2):
                psA = psh.tile([128, AUG], F32, tag="head", name=f"psA{bt}")
                nc.tensor.matmul(psA, ones1, augb_sb, start=True, stop=False)
                for kc in range(4):
                    nc.tensor.matmul(
                        psA, hT[:, kc * B + bt * 128: kc * B + bt * 128 + 128],
                        aug_sb[:, kc * AUG:(kc + 1) * AUG],
                        start=False, stop=(kc == 3))
                psA_sb = sb.tile([128, AUG], F32, tag=f"psAsb{bt}", name=f"psAsb{bt}")
                nc.scalar.copy(psA_sb, psA)
                advs = psA_sb[:, 0:HPC * (M + 1)].rearrange("p (n u) -> p n u", u=M + 1)
                negm = sb.tile([128, HPC], F32, tag=f"negmA{bt}", name=f"negmA{bt}")
                for n in range(HPC):
                    nc.scalar.activation(junkA, advs[:, n, 0:M], COPY,
                                         scale=-1.0 / M,
                                         accum_out=negm[:, n:n + 1])
                tmp = sb.tile([128, HPC], F32, tag=f"tmpA{bt}", name=f"tmpA{bt}")
                nc.gpsimd.tensor_add(tmp, advs[:, :, M], negm)
                q = sb.tile([128, HPC * M], F32, tag=f"qA{bt}", name=f"qA{bt}")
                nc.gpsimd.tensor_tensor(
                    out=q.rearrange("p (n m) -> p n m", m=M),
                    in0=advs[:, :, 0:M],
                    in1=tmp.broadcast_to([128, HPC, M]),
                    op=ADD)
                nc.scalar.dma_start(out_assoc[ts(bt, 128), :], q)
                value_sb.append(psA_sb[:, AUG - 1:AUG])

            # ---- cache head (R-slice) + early row-sum AllGather ----
            ar1 = sb.tile([128, 2], F32, tag="ar1")
            adv_c_sb = []
            for bt in range(2):
                psC = psh.tile([128, RPC], F32, tag="head", name=f"psC{bt}")
                nc.tensor.matmul(psC, ones1, bac_sb, start=True, stop=False)
                for kc in range(4):
                    nc.tensor.matmul(
                        psC, hT[:, kc * B + bt * 128: kc * B + bt * 128 + 128],
                        wac_sb[:, kc * RPC:(kc + 1) * RPC],
                        start=False, stop=(kc == 3))
                t = sb.tile([128, RPC], F32, tag=f"advc{bt}", name=f"advc{bt}")
                nc.scalar.activation(t, psC, COPY, accum_out=ar1[:, bt:bt + 1])
                adv_c_sb.append(t)

            ar1_din = dram.tile([128, 2], F32, tag="ar1_din")
            ar1_dout = dram.tile([NC * 128, 2], F32, tag="ar1_dout")
            nc.gpsimd.dma_start(ar1_din, ar1)
            nc.gpsimd.collective_compute(
                "AllGather", mybir.AluOpType.bypass, replica_groups=RG,
                ins=[ar1_din.opt()], outs=[ar1_dout.opt()])
            rall1 = sb.tile([128, NC * 2], F32, tag="rall1")
            nc.gpsimd.dma_start(rall1, ar1_dout.rearrange("(g p) c -> p g c", p=128))
            rv1 = bass.AP(rall1.tensor, rall1.offset,
                          [rall1.ap[0], [1, 2], [2, NC]])
            sum1 = sb.tile([128, 2], F32, tag="sum1")
            nc.vector.tensor_reduce(sum1, rv1, axis=mybir.AxisListType.X, op=ADD)
            negm1 = sb.tile([128, 2], F32, tag="negm1")
            nc.scalar.activation(negm1, sum1, COPY, scale=-1.0 / R)
            # q_cache finalized + written EARLY (hidden under the wru stream)
            for bt in range(2):
                vm = sb.tile([128, 1], F32, tag=f"vm{bt}", name=f"vm{bt}")
                nc.gpsimd.tensor_add(vm, value_sb[bt], negm1[:, bt:bt + 1])
                qc = sb.tile([128, RPC], F32, tag=f"qc{bt}", name=f"qc{bt}")
                nc.scalar.activation(qc, adv_c_sb[bt], IDENT, bias=vm, scale=1.0)
                nc.scalar.dma_start(out_cache[ts(bt, 128), :], qc)

            # ---- S head: needs the full wru reduction ----
            ar2 = sb.tile([128, 2], F32, tag="ar2")
            s_sb = []
            for bt in range(2):
                psS = psh.tile([128, RPC], F32, tag="head", name=f"psS{bt}")
                nc.tensor.matmul(psS, ones64, bru_sb, start=True, stop=False)
                for kc in range(4):
                    nc.tensor.matmul(
                        psS, hT[:, kc * B + bt * 128: kc * B + bt * 128 + 128],
                        accb[kc], start=False, stop=(kc == 3))
                st = sb.tile([128, RPC], F32, tag=f"ssb{bt}", name=f"ssb{bt}")
                nc.scalar.activation(st, psS, COPY, accum_out=ar2[:, bt:bt + 1])
                s_sb.append(st)

            # ---- tail: S row-sum AllGather, q_rec finalize ----
            ar2_din = dram.tile([128, 2], F32, tag="ar2_din")
            ar2_dout = dram.tile([NC * 128, 2], F32, tag="ar2_dout")
            nc.gpsimd.dma_start(ar2_din, ar2)
            nc.gpsimd.collective_compute(
                "AllGather", mybir.AluOpType.bypass, replica_groups=RG,
                ins=[ar2_din.opt()], outs=[ar2_dout.opt()])
            rall2 = sb.tile([128, NC * 2], F32, tag="rall2")
            nc.gpsimd.dma_start(rall2, ar2_dout.rearrange("(g p) c -> p g c", p=128))
            rv2 = bass.AP(rall2.tensor, rall2.offset,
                          [rall2.ap[0], [1, 2], [2, NC]])
            sum2 = sb.tile([128, 2], F32, tag="sum2")
            nc.vector.tensor_reduce(sum2, rv2, axis=mybir.AxisListType.X, op=ADD)
            negm2 = sb.tile([128, 2], F32, tag="negm2")
            nc.scalar.activation(negm2, sum2, COPY, scale=-1.0 / R)
            for bt in range(2):
                qr = sb.tile([128, RPC], F32, tag=f"qr{bt}", name=f"qr{bt}")
                nc.scalar.activation(qr, s_sb[bt], IDENT,
                                     bias=negm2[:, bt:bt + 1], scale=1.0)
                nc.scalar.dma_start(out_rec[ts(bt, 128), :], qr)

    nc.compile()
    return nc


_CACHED = None


def _get_program():
    global _CACHED
    if _CACHED is None:
        _CACHED = build_program()
    return _CACHED


def make_in_maps(x, W1, b1, W2, b2, Wvc, bvc, Wac, bac, Wvu, bvu, Wau, bau, Wru, bru):
    f = np.float32
    bf = ml_dtypes.bfloat16
    x = np.asarray(x, f)
    W1 = np.asarray(W1, f)
    W2 = np.asarray(W2, f)
    Wac = np.asarray(Wac, f)
    Wru = np.asarray(Wru, f)
    Wau = np.asarray(Wau, f)
    Wvu = np.asarray(Wvu, f)
    Wvc = np.asarray(Wvc, f).reshape(H)
    b1 = np.asarray(b1, f)
    b2 = np.asarray(b2, f)
    bac_v = np.asarray(bac, f)
    bau = np.asarray(bau, f)
    bvu = np.asarray(bvu, f)
    bvc = np.asarray(bvc, f).reshape(1)
    bru_m = np.asarray(bru, f)

    w2p = np.ascontiguousarray(
        W2.reshape(4, 128, H).transpose(1, 0, 2)).reshape(128, 4 * H).astype(bf)
    b2cp = np.ascontiguousarray(b2.reshape(4, 128).T)

    in_maps = []
    for c in range(NC):
        k0 = c * KPC_RAW
        xs = np.zeros((KPC, B), f)
        xs[:KPC_RAW] = x[:, k0:k0 + KPC_RAW].T
        xtp = np.ascontiguousarray(
            xs.reshape(KCH, 128, B).transpose(1, 0, 2)).reshape(128, KCH * B).astype(bf)
        w1s = np.zeros((KPC, H), f)
        w1s[:KPC_RAW] = W1[k0:k0 + KPC_RAW]
        w1p = np.ascontiguousarray(
            w1s.reshape(KCH, 128, H).transpose(1, 0, 2)).reshape(128, KCH * H).astype(bf)

        r0 = c * RPC
        h0 = c * HPC
        # wru supertiles, head-blocked: [kc, grp, p, gi*RPC+r]
        #   = Wru[grp*GRP+gi, kc*128+p, r0+r]
        ws = Wru[:, :, r0:r0 + RPC]                          # [64, 512, 500]
        a = ws.reshape(NGRP, GRP, 4, 128, RPC)               # [grp, gi, kc, p, r]
        wrup_ = np.ascontiguousarray(a.transpose(2, 0, 3, 1, 4)).reshape(
            4, NGRP, 128, GRP * RPC).astype(bf)

        wacp = np.ascontiguousarray(
            Wac[:, r0:r0 + RPC].reshape(4, 128, RPC).transpose(1, 0, 2)).reshape(
            128, 4 * RPC).astype(bf)

        aug = np.zeros((4, 128, AUG), f)
        ag = aug[:, :, :HPC * (M + 1)].reshape(4, 128, HPC, M + 1)
        ag[:, :, :, :M] = Wau[h0:h0 + HPC].reshape(HPC, 4, 128, M).transpose(1, 2, 0, 3)
        ag[:, :, :, M] = Wvu[h0:h0 + HPC].reshape(HPC, 4, 128).transpose(1, 2, 0)
        aug[:, :, AUG - 1] = Wvc.reshape(4, 128)
        augp = np.ascontiguousarray(aug.transpose(1, 0, 2)).reshape(128, 4 * AUG).astype(bf)
        augb = np.concatenate([
            np.concatenate([bau[h0:h0 + HPC], bvu[h0:h0 + HPC, None]], axis=1).reshape(-1),
            bvc]).reshape(1, AUG).astype(bf)

        m = {
            "xt": xtp, "w1": w1p, "w2": w2p,
            "b1s": np.ascontiguousarray(b1[c * HS:(c + 1) * HS].reshape(HS, 1)),
            "b2c": b2cp,
            "wac": wacp,
            "bac": np.ascontiguousarray(bac_v[r0:r0 + RPC].reshape(1, RPC)).astype(bf),
            "wru": wrup_,
            "bru": np.ascontiguousarray(bru_m[:, r0:r0 + RPC]).astype(bf),
            "aug_w": augp, "aug_b": augb,
        }
        in_maps.append(m)
    return in_maps


def assemble(results):
    q = np.empty((B, 2 * R + NH * M), np.float32)
    for c in range(NC):
        r0 = c * RPC
        a0 = c * HPC * M
        q[:, r0:r0 + RPC] = results[c]["out_cache"]
        q[:, R + r0:R + r0 + RPC] = results[c]["out_rec"]
        q[:, 2 * R + a0:2 * R + a0 + HPC * M] = results[c]["out_assoc"]
    return q


def run(in_maps, **kw):
    nc = _get_program()
    return bass_utils.run_bass_kernel_spmd(nc, in_maps, core_ids=list(range(NC)), **kw)


def kernel(**inputs):
    in_maps = make_in_maps(**{k: np.asarray(v) for k, v in inputs.items()})
    res = run(in_maps)
    return assemble(res.results)
